# revision 29
# baseline (speedup 1.0000x reference)
"""GQA attention (B=2,S=2048,D=4096,H=32,KVH=8,HD=128, RoPE, causal) on 8 TRN2 cores.

Sharding: tensor-parallel over heads. Core c owns q-heads 4c..4c+3 and kv-head c.
Per core: QKV projections (bf16 x / bf16 weights -> f32 psum), RoPE (f32),
transposed-scores causal attention in f32r (scores^T = K^T-tiles x Q^T so the
softmax denominator comes from a ones-matmul and P^T feeds PV directly),
pipelined AllGather of normalized attention output (bf16, one AG per
(batch, 512-query chunk)), and the core's 512-column shard of Wo (bf16)
interleaved with the other batch's projections / later attention chunks.
Host concatenates the 8 column shards.

Self-contained: hardcodes all shapes; no file reads.
"""
import math

import numpy as np
import ml_dtypes

import concourse.mybir as mybir
import concourse.tile as tile
from concourse import bacc
from concourse.bass_utils import run_bass_kernel_spmd

N_CORES = 8
B, S, D = 2, 2048, 4096
H, KVH, HD = 32, 8, 128
HL = H // N_CORES          # 4 local q heads
TOK = B * S

F32 = mybir.dt.float32
F32R = mybir.dt.float32r
BF16 = mybir.dt.bfloat16

TBLK = 256                 # projection token block
QBLK = 512                 # attention query block / AG chunk / Wo token block
KC = D // 128              # 32 contraction chunks
NTB = S // TBLK            # 8 token blocks per batch
NQB = S // QBLK            # 4 query blocks per batch


def tf32_round(x: np.ndarray) -> np.ndarray:
    u = np.ascontiguousarray(x, dtype=np.float32).view(np.uint32)
    r = (u + np.uint32(0x0FFF) + ((u >> np.uint32(13)) & np.uint32(1))) & np.uint32(0xFFFFE000)
    return r.view(np.float32)


def build():
    nc = bacc.Bacc("TRN2", target_bir_lowering=False, debug=False, num_devices=N_CORES)

    xt = nc.declare_dram_parameter("xt", [B, D, S], BF16, isOutput=False)
    wq = nc.declare_dram_parameter("wq", [D, HL * HD], BF16, isOutput=False)
    wk = nc.declare_dram_parameter("wk", [D, HD], BF16, isOutput=False)
    wv = nc.declare_dram_parameter("wv", [D, HD], BF16, isOutput=False)
    wo = nc.declare_dram_parameter("wo", [D, 512], BF16, isOutput=False)
    cosf = nc.declare_dram_parameter("cosf", [128, S], BF16, isOutput=False)
    sinf = nc.declare_dram_parameter("sinf", [128, S], BF16, isOutput=False)
    masks = nc.declare_dram_parameter("masks", [4, 128, QBLK], BF16, isOutput=False)
    ident = nc.declare_dram_parameter("ident", [128, 128], F32R, isOutput=False)
    ones = nc.declare_dram_parameter("ones", [128, 128], F32R, isOutput=False)
    out = nc.declare_dram_parameter("out", [512, TOK], F32, isOutput=True)

    with tile.TileContext(nc) as tc:
        with (
            tc.tile_pool(name="consts", bufs=1) as consts,
            tc.tile_pool(name="wop", bufs=1) as wop,
            tc.tile_pool(name="wps", bufs=1, space="PSUM") as wps,
            tc.tile_pool(name="dram", bufs=1, space="DRAM") as dram,
        ):
            dummy_in = dram.tile([1, 64], F32, name="dummy_in")
            dummy_out = dram.tile([N_CORES, 64], F32, name="dummy_out", addr_space="Shared")
            dummy_bk = dram.tile([1, 64], F32, name="dummy_bk")
            nc.gpsimd.collective_compute(
                "AllGather", mybir.AluOpType.bypass,
                replica_groups=[list(range(N_CORES))],
                ins=[dummy_in.opt()], outs=[dummy_out.opt()])
            cos_sb = consts.tile([128, S], BF16)
            sin_sb = consts.tile([128, S], BF16)
            mask_sb = consts.tile([128, 4 * QBLK], BF16)
            id_sb = consts.tile([128, 128], F32R)
            ones_sb = consts.tile([128, 128], F32R)
            nc.scalar.dma_start(cos_sb[:], cosf[:])
            nc.scalar.dma_start(sin_sb[:], sinf[:])
            nc.scalar.dma_start(mask_sb.rearrange("p (i n) -> p i n", i=4),
                              masks.rearrange("i p n -> p i n"))
            nc.scalar.dma_start(id_sb[:], ident[:])
            nc.scalar.dma_start(ones_sb[:], ones[:])
            wo_sb = wop.tile([128, KC, 512], BF16)
            wq_sb = wop.tile([128, KC, HL * HD], BF16, name="wq_sb")
            wk_sb = wop.tile([128, KC, HD], BF16, name="wk_sb")
            wv_sb = wop.tile([128, KC, HD], BF16, name="wv_sb")
            def load_wq_quarter(q4):
                cs = slice(8 * q4, 8 * (q4 + 1))
                rs = slice(1024 * q4, 1024 * (q4 + 1))
                nc.sync.dma_start(wq_sb[:, cs, :],
                                  wq[rs, :].rearrange("(c p) m -> p c m", p=128))
                nc.gpsimd.dma_start(wk_sb[:, cs, :],
                                    wk[rs, :].rearrange("(c p) m -> p c m", p=128))
                nc.gpsimd.dma_start(wv_sb[:, cs, :],
                                    wv[rs, :].rearrange("(c p) m -> p c m", p=128))

            def load_wo():
                for q4 in range(4):
                    cs = slice(8 * q4, 8 * (q4 + 1))
                    rs = slice(1024 * q4, 1024 * (q4 + 1))
                    nc.sync.dma_start(wo_sb[:, cs, :],
                                      wo[rs, :].rearrange("(c p) m -> p c m", p=128))

            qt_d = [dram.tile([HL * 128, S], F32R, name=f"qt_d{b}") for b in range(B)]
            kt_d = [dram.tile([128, S], F32R, name=f"kt_d{b}") for b in range(B)]
            v_d = [dram.tile([S, 128], F32R, name=f"v_d{b}") for b in range(B)]
            CH = [(b, qb) for b in range(B) for qb in range(NQB)]
            agin_d = {c: dram.tile([512, QBLK], BF16, name=f"agin_{c[0]}_{c[1]}") for c in CH}
            agout_d = {c: dram.tile([512 * N_CORES, QBLK], BF16, name=f"agout_{c[0]}_{c[1]}",
                                    addr_space="Shared") for c in CH}

            rj_cache = {}

            def wo_prefetch(b, qb):
                rjs = []
                for jg in range(KC // 4):
                    rj = wop.tile([128, 4, 512], BF16, name=f"rj_{b}_{qb}_{jg}",
                                  tag="rj", bufs=9)
                    nc.sync.dma_start(
                        rj[:],
                        agout_d[(b, qb)][512 * jg:512 * (jg + 1), :]
                        .rearrange("(i p) t -> p i t", p=128))
                    rjs.append(rj)
                rj_cache[(b, qb)] = rjs

            def wo_chunk(b, qb):
                rjs = rj_cache.pop((b, qb))
                t0 = qb * QBLK
                for dc in range(4):
                    wo_ps = wps.tile([128, 512], F32, name=f"wo_{b}_{qb}_{dc}",
                                     tag="wo", bufs=2)
                    for jc in range(KC):
                        nc.tensor.matmul(wo_ps[:], wo_sb[:, jc, 128 * dc:128 * (dc + 1)],
                                         rjs[jc // 4][:, jc % 4, :],
                                         start=(jc == 0), stop=(jc == KC - 1))
                    osb = wop.tile([128, 512], F32, name=f"o_{b}_{qb}_{dc}",
                                   tag="osb", bufs=2)
                    nc.scalar.copy(osb[:], wo_ps[:])
                    nc.scalar.dma_start(out[128 * dc:128 * (dc + 1),
                                            b * S + t0:b * S + t0 + 512], osb[:])

            def proj_phase(b, interleave=None):
                # interleave: dict tb -> list of callables emitted after that tblk
                with (
                    tc.tile_pool(name=f"xtp{b}", bufs=1) as xtp,
                    tc.tile_pool(name=f"pevac{b}", bufs=1) as pevac,
                ):
                    pps = aps_pool
                    for tb in range(NTB):
                        t0 = tb * TBLK
                        xg = []
                        for g in range(KC // 4):
                            if b == 0 and tb == 0 and g % 2 == 0:
                                load_wq_quarter(g // 2)
                            xt_t = xtp.tile([128, 4, TBLK], BF16, name=f"xt_{b}_{tb}_{g}",
                                            tag="xt", bufs=9)
                            (nc.sync if (b == 0 and tb == 0) else nc.gpsimd).dma_start(
                                xt_t[:],
                                xt[b, 512 * g:512 * (g + 1), t0:t0 + TBLK]
                                .rearrange("(i p) t -> p i t", p=128))
                            xg.append(xt_t)

                        def xts(c):
                            return xg[c // 4][:, c % 4, :]

                        def proj_rope(w_sb, h, dst, dst_sl):
                            ps_t = pps.tile([128, TBLK], F32, name=f"ps_{b}_{tb}_{h}",
                                            tag="sc", bufs=4)
                            for c in range(KC):
                                nc.tensor.matmul(ps_t[:], w_sb[:, c, 128 * h:128 * (h + 1)],
                                                 xts(c), start=(c == 0), stop=(c == KC - 1))
                            ev = pevac.tile([128, TBLK], F32, name=f"ev_{b}_{tb}_{h}",
                                            tag="ev", bufs=3)
                            nc.scalar.copy(ev[:], ps_t[:])
                            rot = pevac.tile([128, TBLK], F32, name=f"rot_{b}_{tb}_{h}",
                                             tag="rot", bufs=3)
                            nc.scalar.copy(rot[0:64, :], ev[64:128, :])
                            nc.scalar.copy(rot[64:128, :], ev[0:64, :])
                            nc.vector.tensor_mul(ev[:], ev[:], cos_sb[:, t0:t0 + TBLK])
                            nc.vector.tensor_mul(rot[:], rot[:], sin_sb[:, t0:t0 + TBLK])
                            ro = pevac.tile([128, TBLK], F32R, name=f"ro_{b}_{tb}_{h}",
                                            tag="ro", bufs=3)
                            nc.vector.tensor_add(ro[:], ev[:], rot[:])
                            nc.gpsimd.dma_start(dst[dst_sl], ro[:])

                        for h in range(HL):
                            proj_rope(wq_sb, h, qt_d[b],
                                      (slice(128 * h, 128 * (h + 1)), slice(t0, t0 + TBLK)))
                        proj_rope(wk_sb, 0, kt_d[b], (slice(0, 128), slice(t0, t0 + TBLK)))

                        ps_v = pps.tile([128, TBLK], F32, name=f"psv_{b}_{tb}", tag="sc", bufs=4)
                        for c in range(KC):
                            nc.tensor.matmul(ps_v[:], wv_sb[:, c, :], xts(c),
                                             start=(c == 0), stop=(c == KC - 1))
                        vt_sb = pevac.tile([128, TBLK], F32R, name=f"vt_{b}_{tb}",
                                           tag="vt", bufs=3)
                        nc.scalar.copy(vt_sb[:], ps_v[:])
                        for i in range(TBLK // 128):
                            ps_tr = pps.tile([128, 128], F32R, name=f"ptr_{b}_{tb}_{i}",
                                             tag=("at" if i % 2 == 0 else "den"), bufs=1)
                            nc.tensor.transpose(ps_tr[:], vt_sb[:, 128 * i:128 * (i + 1)],
                                                id_sb[:])
                            vn = pevac.tile([128, 128], F32R, name=f"vn_{b}_{tb}_{i}",
                                            tag="vn", bufs=4)
                            nc.vector.tensor_copy(vn[:], ps_tr[:])
                            nc.gpsimd.dma_start(v_d[b][t0 + 128 * i:t0 + 128 * (i + 1), :],
                                                vn[:])

                        if interleave and tb in interleave:
                            for fn in interleave[tb]:
                                fn()

            def att_phase(b, post=None):
                # post: dict qb -> list of callables emitted after that chunk
                if True:
                    att, aps = att_pool, aps_pool
                    ktg = []
                    for q4 in range(4):
                        kt_t = att.tile([128, 512], F32R, name=f"kt_{b}_{q4}",
                                        tag="kt", bufs=4)
                        nc.sync.dma_start(kt_t[:], kt_d[b][:, 512 * q4:512 * (q4 + 1)])
                        ktg.append(kt_t)
                    vg = []
                    for g in range(4):
                        v_sb = att.tile([128, 4, 128], F32R, name=f"v_{b}_{g}", tag="v", bufs=5)
                        nc.sync.dma_start(
                            v_sb[:],
                            v_d[b][512 * g:512 * (g + 1), :].rearrange("(i p) t -> p i t", p=128))
                        vg.append(v_sb)

                    for qb in range(NQB):
                        q0 = qb * QBLK
                        nkb = 4 * qb + 4
                        for h in range(HL):
                            qt_sb = att.tile([128, QBLK], F32R, name=f"qt_{b}_{qb}_{h}",
                                             tag="qt", bufs=6)
                            nc.sync.dma_start(qt_sb[:],
                                              qt_d[b][128 * h:128 * (h + 1), q0:q0 + QBLK])
                            at_ps = aps.tile([128, QBLK], F32, name=f"at_{b}_{qb}_{h}",
                                             tag="at", bufs=1)
                            den_ps = aps.tile([128, QBLK], F32, name=f"den_{b}_{qb}_{h}",
                                              tag="den", bufs=1)
                            for kb in range(nkb):
                                i = kb - 4 * qb
                                qoff = 128 * i if i > 0 else 0
                                qn = QBLK - qoff
                                sc_ps = aps.tile([128, QBLK], F32, name=f"sc_{b}_{qb}_{h}_{kb}",
                                                 tag="sc", bufs=4)
                                nc.tensor.matmul(sc_ps[:, :qn],
                                                 ktg[kb // 4][:, 128 * (kb % 4):128 * (kb % 4 + 1)],
                                                 qt_sb[:, qoff:], start=True, stop=True)
                                pt_sb = att.tile([128, QBLK], F32R,
                                                 name=f"pt_{b}_{qb}_{h}_{kb}", tag="pt", bufs=4)
                                nc.scalar.activation(pt_sb[:, qoff:], sc_ps[:, :qn],
                                                     mybir.ActivationFunctionType.Exp)
                                if i >= 0:
                                    nc.vector.tensor_mul(
                                        pt_sb[:, qoff:], pt_sb[:, qoff:],
                                        mask_sb[:, QBLK * i + qoff:QBLK * (i + 1)])
                                nc.tensor.matmul(at_ps[:, qoff:], vg[kb // 4][:, kb % 4, :],
                                                 pt_sb[:, qoff:],
                                                 start=(kb == 0), stop=(kb == nkb - 1))
                                nc.tensor.matmul(den_ps[:, qoff:], ones_sb[:],
                                                 pt_sb[:, qoff:],
                                                 start=(kb == 0), stop=(kb == nkb - 1))
                            recip = att.tile([128, QBLK], F32, name=f"rc_{b}_{qb}_{h}",
                                             tag="rc", bufs=1)
                            nc.vector.reciprocal_approx_fast(recip[:], den_ps[:])
                            an_sb = att.tile([128, QBLK], BF16, name=f"an_{b}_{qb}_{h}",
                                             tag="an", bufs=2)
                            nc.vector.tensor_mul(an_sb[:], at_ps[:], recip[:])
                            nc.gpsimd.dma_start(agin_d[(b, qb)][128 * h:128 * (h + 1), :],
                                                an_sb[:])
                        nc.gpsimd.collective_compute(
                            "AllGather",
                            mybir.AluOpType.bypass,
                            replica_groups=[list(range(N_CORES))],
                            ins=[agin_d[(b, qb)].opt()],
                            outs=[agout_d[(b, qb)].opt()],
                        )
                        if post and qb in post:
                            for fn in post[qb]:
                                fn()

            # pipeline: proj(0) | attn(0) | proj(1) w/ wo(0,*) | attn(1) | wo(1,*)
            with (
                tc.tile_pool(name="att", bufs=1) as att_pool,
                tc.tile_pool(name="aps", bufs=1, space="PSUM") as aps_pool,
            ):
                proj_phase(0, interleave={0: [load_wo]})
                dummy_sb = consts.tile([N_CORES, 64], F32, name="dummy_sb")
                nc.gpsimd.dma_start(dummy_sb[:], dummy_out[:])
                nc.gpsimd.dma_start(dummy_bk[:], dummy_sb[0:1, :])
                att_phase(0)
                proj_phase(1, interleave={
                    0: [lambda: wo_prefetch(0, 0), lambda: wo_prefetch(0, 1),
                        lambda: wo_chunk(0, 0)],
                    2: [lambda: wo_prefetch(0, 2), lambda: wo_chunk(0, 1)],
                    4: [lambda: wo_prefetch(0, 3), lambda: wo_chunk(0, 2)],
                    6: [lambda: wo_chunk(0, 3)],
                })
                att_phase(1, post={
                    1: [lambda: wo_prefetch(1, 0)],
                    2: [lambda: wo_prefetch(1, 1)],
                    3: [lambda: wo_chunk(1, 0), lambda: wo_prefetch(1, 2),
                        lambda: wo_chunk(1, 1), lambda: wo_prefetch(1, 3),
                        lambda: wo_chunk(1, 2), lambda: wo_chunk(1, 3)],
                })
    nc.compile()
    return nc


_NC_CACHE = None
_LAST_IN_MAPS = None


def kernel(x, freqs_cos, freqs_sin, Wq, Wk, Wv, Wo):
    global _NC_CACHE, _LAST_IN_MAPS
    x = np.asarray(x, dtype=np.float32)
    freqs_cos = np.asarray(freqs_cos, dtype=np.float32)
    freqs_sin = np.asarray(freqs_sin, dtype=np.float32)
    Wq = np.asarray(Wq, dtype=np.float32)
    Wk = np.asarray(Wk, dtype=np.float32)
    Wv = np.asarray(Wv, dtype=np.float32)
    Wo = np.asarray(Wo, dtype=np.float32)

    # de-interleave rope pairs within each head: [0,2,...,126, 1,3,...,127]
    perm = np.concatenate([np.arange(0, HD, 2), np.arange(1, HD, 2)])
    scale = 1.0 / math.sqrt(HD)

    xt = x.transpose(0, 2, 1).astype(ml_dtypes.bfloat16)       # [B, D, S]

    cosf = np.empty((128, S), np.float32)
    sinf = np.empty((128, S), np.float32)
    cosf[0:64] = freqs_cos.T
    cosf[64:128] = freqs_cos.T
    sinf[0:64] = -freqs_sin.T
    sinf[64:128] = freqs_sin.T

    mk = np.zeros((4, 128, QBLK), np.float32)
    for i in range(4):
        k_idx = np.arange(128)[:, None]
        j_idx = np.arange(QBLK)[None, :]
        mk[i] = (j_idx >= k_idx + 128 * i).astype(np.float32)

    ident = np.eye(128, dtype=np.float32)
    ones = np.ones((128, 128), np.float32)

    in_maps = []
    for c in range(N_CORES):
        wq_c = Wq[:, 512 * c:512 * (c + 1)].reshape(D, HL, HD)[:, :, perm].reshape(D, HL * HD)
        wk_c = Wk[:, HD * c:HD * (c + 1)][:, perm]
        wv_c = Wv[:, HD * c:HD * (c + 1)]
        wo_c = Wo[:, 512 * c:512 * (c + 1)]
        in_maps.append({
            "xt": xt,
            "wq": (wq_c * scale).astype(ml_dtypes.bfloat16),
            "wk": wk_c.astype(ml_dtypes.bfloat16),
            "wv": wv_c.astype(ml_dtypes.bfloat16),
            "wo": wo_c.astype(ml_dtypes.bfloat16),
            "cosf": cosf.astype(ml_dtypes.bfloat16),
            "sinf": sinf.astype(ml_dtypes.bfloat16),
            "masks": mk.astype(ml_dtypes.bfloat16),
            "ident": ident,
            "ones": ones,
        })

    if _NC_CACHE is None:
        _NC_CACHE = build()
    _LAST_IN_MAPS = in_maps
    res = run_bass_kernel_spmd(_NC_CACHE, in_maps, core_ids=list(range(N_CORES)))

    full = np.empty((B, S, D), np.float32)
    for c in range(N_CORES):
        oc = res.results[c]["out"]                             # [512, B*S]
        full[:, :, 512 * c:512 * (c + 1)] = oc.reshape(512, B, S).transpose(1, 2, 0)
    return full


# revision 30
# speedup vs baseline: 1.0046x; 1.0046x over previous
"""GQA attention (B=2,S=2048,D=4096,H=32,KVH=8,HD=128, RoPE, causal) on 8 TRN2 cores.

Sharding: tensor-parallel over heads. Core c owns q-heads 4c..4c+3 and kv-head c.
Per core: QKV projections (bf16 x / bf16 weights -> f32 psum), RoPE (f32),
transposed-scores causal attention in f32r (scores^T = K^T-tiles x Q^T so the
softmax denominator comes from a ones-matmul and P^T feeds PV directly),
pipelined AllGather of normalized attention output (bf16, one AG per
(batch, 512-query chunk)), and the core's 512-column shard of Wo (bf16)
interleaved with the other batch's projections / later attention chunks.
Host concatenates the 8 column shards.

Self-contained: hardcodes all shapes; no file reads.
"""
import math

import numpy as np
import ml_dtypes

import concourse.mybir as mybir
import concourse.tile as tile
from concourse import bacc
from concourse.bass_utils import run_bass_kernel_spmd

N_CORES = 8
B, S, D = 2, 2048, 4096
H, KVH, HD = 32, 8, 128
HL = H // N_CORES          # 4 local q heads
TOK = B * S

F32 = mybir.dt.float32
F32R = mybir.dt.float32r
BF16 = mybir.dt.bfloat16

TBLK = 256                 # projection token block
QBLK = 512                 # attention query block / AG chunk / Wo token block
KC = D // 128              # 32 contraction chunks
NTB = S // TBLK            # 8 token blocks per batch
NQB = S // QBLK            # 4 query blocks per batch


def tf32_round(x: np.ndarray) -> np.ndarray:
    u = np.ascontiguousarray(x, dtype=np.float32).view(np.uint32)
    r = (u + np.uint32(0x0FFF) + ((u >> np.uint32(13)) & np.uint32(1))) & np.uint32(0xFFFFE000)
    return r.view(np.float32)


def build():
    nc = bacc.Bacc("TRN2", target_bir_lowering=False, debug=False, num_devices=N_CORES)

    xt = nc.declare_dram_parameter("xt", [B, D, S], BF16, isOutput=False)
    wq = nc.declare_dram_parameter("wq", [D, HL * HD], BF16, isOutput=False)
    wk = nc.declare_dram_parameter("wk", [D, HD], BF16, isOutput=False)
    wv = nc.declare_dram_parameter("wv", [D, HD], BF16, isOutput=False)
    wo = nc.declare_dram_parameter("wo", [D, 512], BF16, isOutput=False)
    cosf = nc.declare_dram_parameter("cosf", [128, S], BF16, isOutput=False)
    sinf = nc.declare_dram_parameter("sinf", [128, S], BF16, isOutput=False)
    masks = nc.declare_dram_parameter("masks", [4, 128, QBLK], BF16, isOutput=False)
    ident = nc.declare_dram_parameter("ident", [128, 128], F32R, isOutput=False)
    ones = nc.declare_dram_parameter("ones", [128, 128], F32R, isOutput=False)
    out = nc.declare_dram_parameter("out", [512, TOK], F32, isOutput=True)

    with tile.TileContext(nc) as tc:
        with (
            tc.tile_pool(name="consts", bufs=1) as consts,
            tc.tile_pool(name="wop", bufs=1) as wop,
            tc.tile_pool(name="wps", bufs=1, space="PSUM") as wps,
            tc.tile_pool(name="dram", bufs=1, space="DRAM") as dram,
        ):
            dummy_in = dram.tile([1, 64], F32, name="dummy_in")
            dummy_out = dram.tile([N_CORES, 64], F32, name="dummy_out", addr_space="Shared")
            dummy_bk = dram.tile([1, 64], F32, name="dummy_bk")
            nc.gpsimd.collective_compute(
                "AllGather", mybir.AluOpType.bypass,
                replica_groups=[list(range(N_CORES))],
                ins=[dummy_in.opt()], outs=[dummy_out.opt()])
            cos_sb = consts.tile([128, S], BF16)
            sin_sb = consts.tile([128, S], BF16)
            mask_sb = consts.tile([128, 4 * QBLK], BF16)
            id_sb = consts.tile([128, 128], F32R)
            ones_sb = consts.tile([128, 128], F32R)
            nc.scalar.dma_start(cos_sb[:], cosf[:])
            nc.scalar.dma_start(sin_sb[:], sinf[:])
            nc.scalar.dma_start(mask_sb.rearrange("p (i n) -> p i n", i=4),
                              masks.rearrange("i p n -> p i n"))
            nc.scalar.dma_start(id_sb[:], ident[:])
            nc.scalar.dma_start(ones_sb[:], ones[:])
            wo_sb = wop.tile([128, KC, 512], BF16)
            wq_sb = wop.tile([128, KC, HL * HD], BF16, name="wq_sb")
            wk_sb = wop.tile([128, KC, HD], BF16, name="wk_sb")
            wv_sb = wop.tile([128, KC, HD], BF16, name="wv_sb")
            def load_wq_quarter(q4):
                if q4 == 0:
                    for e in range(4):
                        cs = slice(2 * e, 2 * (e + 1))
                        rs = slice(256 * e, 256 * (e + 1))
                        nc.sync.dma_start(wq_sb[:, cs, :],
                                          wq[rs, :].rearrange("(c p) m -> p c m", p=128))
                else:
                    cs = slice(8 * q4, 8 * (q4 + 1))
                    rs = slice(1024 * q4, 1024 * (q4 + 1))
                    nc.sync.dma_start(wq_sb[:, cs, :],
                                      wq[rs, :].rearrange("(c p) m -> p c m", p=128))
                cs = slice(8 * q4, 8 * (q4 + 1))
                rs = slice(1024 * q4, 1024 * (q4 + 1))
                nc.gpsimd.dma_start(wk_sb[:, cs, :],
                                    wk[rs, :].rearrange("(c p) m -> p c m", p=128))
                nc.gpsimd.dma_start(wv_sb[:, cs, :],
                                    wv[rs, :].rearrange("(c p) m -> p c m", p=128))

            def load_wo():
                for q4 in range(4):
                    cs = slice(8 * q4, 8 * (q4 + 1))
                    rs = slice(1024 * q4, 1024 * (q4 + 1))
                    nc.sync.dma_start(wo_sb[:, cs, :],
                                      wo[rs, :].rearrange("(c p) m -> p c m", p=128))

            qt_d = [dram.tile([HL * 128, S], F32R, name=f"qt_d{b}") for b in range(B)]
            kt_d = [dram.tile([128, S], F32R, name=f"kt_d{b}") for b in range(B)]
            v_d = [dram.tile([S, 128], F32R, name=f"v_d{b}") for b in range(B)]
            CH = [(b, qb) for b in range(B) for qb in range(NQB)]
            agin_d = {c: dram.tile([512, QBLK], BF16, name=f"agin_{c[0]}_{c[1]}") for c in CH}
            agout_d = {c: dram.tile([512 * N_CORES, QBLK], BF16, name=f"agout_{c[0]}_{c[1]}",
                                    addr_space="Shared") for c in CH}

            rj_cache = {}

            def wo_prefetch(b, qb):
                rjs = []
                for jg in range(KC // 4):
                    rj = wop.tile([128, 4, 512], BF16, name=f"rj_{b}_{qb}_{jg}",
                                  tag="rj", bufs=9)
                    nc.sync.dma_start(
                        rj[:],
                        agout_d[(b, qb)][512 * jg:512 * (jg + 1), :]
                        .rearrange("(i p) t -> p i t", p=128))
                    rjs.append(rj)
                rj_cache[(b, qb)] = rjs

            def wo_chunk(b, qb):
                rjs = rj_cache.pop((b, qb))
                t0 = qb * QBLK
                for dc in range(4):
                    wo_ps = wps.tile([128, 512], F32, name=f"wo_{b}_{qb}_{dc}",
                                     tag="wo", bufs=2)
                    for jc in range(KC):
                        nc.tensor.matmul(wo_ps[:], wo_sb[:, jc, 128 * dc:128 * (dc + 1)],
                                         rjs[jc // 4][:, jc % 4, :],
                                         start=(jc == 0), stop=(jc == KC - 1))
                    osb = wop.tile([128, 512], F32, name=f"o_{b}_{qb}_{dc}",
                                   tag="osb", bufs=2)
                    nc.scalar.copy(osb[:], wo_ps[:])
                    nc.scalar.dma_start(out[128 * dc:128 * (dc + 1),
                                            b * S + t0:b * S + t0 + 512], osb[:])

            def proj_phase(b, interleave=None):
                # interleave: dict tb -> list of callables emitted after that tblk
                with (
                    tc.tile_pool(name=f"xtp{b}", bufs=1) as xtp,
                    tc.tile_pool(name=f"pevac{b}", bufs=1) as pevac,
                ):
                    pps = aps_pool
                    for tb in range(NTB):
                        t0 = tb * TBLK
                        xg = []
                        for g in range(KC // 4):
                            if b == 0 and tb == 0 and g % 2 == 0:
                                load_wq_quarter(g // 2)
                            xt_t = xtp.tile([128, 4, TBLK], BF16, name=f"xt_{b}_{tb}_{g}",
                                            tag="xt", bufs=9)
                            eng = nc.sync if (b == 0 and tb == 0) else nc.gpsimd
                            if b == 0 and tb == 0 and g == 0:
                                for ii in range(4):
                                    eng.dma_start(
                                        xt_t[:, ii, :],
                                        xt[0, 128 * ii:128 * (ii + 1), t0:t0 + TBLK])
                            else:
                                eng.dma_start(
                                    xt_t[:],
                                    xt[b, 512 * g:512 * (g + 1), t0:t0 + TBLK]
                                    .rearrange("(i p) t -> p i t", p=128))
                            xg.append(xt_t)

                        def xts(c):
                            return xg[c // 4][:, c % 4, :]

                        def proj_rope(w_sb, h, dst, dst_sl):
                            ps_t = pps.tile([128, TBLK], F32, name=f"ps_{b}_{tb}_{h}",
                                            tag="sc", bufs=4)
                            for c in range(KC):
                                nc.tensor.matmul(ps_t[:], w_sb[:, c, 128 * h:128 * (h + 1)],
                                                 xts(c), start=(c == 0), stop=(c == KC - 1))
                            ev = pevac.tile([128, TBLK], F32, name=f"ev_{b}_{tb}_{h}",
                                            tag="ev", bufs=3)
                            nc.scalar.copy(ev[:], ps_t[:])
                            rot = pevac.tile([128, TBLK], F32, name=f"rot_{b}_{tb}_{h}",
                                             tag="rot", bufs=3)
                            nc.scalar.copy(rot[0:64, :], ev[64:128, :])
                            nc.scalar.copy(rot[64:128, :], ev[0:64, :])
                            nc.vector.tensor_mul(ev[:], ev[:], cos_sb[:, t0:t0 + TBLK])
                            nc.vector.tensor_mul(rot[:], rot[:], sin_sb[:, t0:t0 + TBLK])
                            ro = pevac.tile([128, TBLK], F32R, name=f"ro_{b}_{tb}_{h}",
                                            tag="ro", bufs=3)
                            nc.vector.tensor_add(ro[:], ev[:], rot[:])
                            nc.gpsimd.dma_start(dst[dst_sl], ro[:])

                        for h in range(HL):
                            proj_rope(wq_sb, h, qt_d[b],
                                      (slice(128 * h, 128 * (h + 1)), slice(t0, t0 + TBLK)))
                        proj_rope(wk_sb, 0, kt_d[b], (slice(0, 128), slice(t0, t0 + TBLK)))

                        ps_v = pps.tile([128, TBLK], F32, name=f"psv_{b}_{tb}", tag="sc", bufs=4)
                        for c in range(KC):
                            nc.tensor.matmul(ps_v[:], wv_sb[:, c, :], xts(c),
                                             start=(c == 0), stop=(c == KC - 1))
                        vt_sb = pevac.tile([128, TBLK], F32R, name=f"vt_{b}_{tb}",
                                           tag="vt", bufs=3)
                        nc.scalar.copy(vt_sb[:], ps_v[:])
                        for i in range(TBLK // 128):
                            ps_tr = pps.tile([128, 128], F32R, name=f"ptr_{b}_{tb}_{i}",
                                             tag=("at" if i % 2 == 0 else "den"), bufs=1)
                            nc.tensor.transpose(ps_tr[:], vt_sb[:, 128 * i:128 * (i + 1)],
                                                id_sb[:])
                            vn = pevac.tile([128, 128], F32R, name=f"vn_{b}_{tb}_{i}",
                                            tag="vn", bufs=4)
                            nc.vector.tensor_copy(vn[:], ps_tr[:])
                            nc.gpsimd.dma_start(v_d[b][t0 + 128 * i:t0 + 128 * (i + 1), :],
                                                vn[:])

                        if interleave and tb in interleave:
                            for fn in interleave[tb]:
                                fn()

            def att_phase(b, post=None):
                # post: dict qb -> list of callables emitted after that chunk
                if True:
                    att, aps = att_pool, aps_pool
                    ktg = []
                    for q4 in range(4):
                        kt_t = att.tile([128, 512], F32R, name=f"kt_{b}_{q4}",
                                        tag="kt", bufs=4)
                        nc.sync.dma_start(kt_t[:], kt_d[b][:, 512 * q4:512 * (q4 + 1)])
                        ktg.append(kt_t)
                    vg = []
                    for g in range(4):
                        v_sb = att.tile([128, 4, 128], F32R, name=f"v_{b}_{g}", tag="v", bufs=5)
                        nc.sync.dma_start(
                            v_sb[:],
                            v_d[b][512 * g:512 * (g + 1), :].rearrange("(i p) t -> p i t", p=128))
                        vg.append(v_sb)

                    for qb in range(NQB):
                        q0 = qb * QBLK
                        nkb = 4 * qb + 4
                        for h in range(HL):
                            qt_sb = att.tile([128, QBLK], F32R, name=f"qt_{b}_{qb}_{h}",
                                             tag="qt", bufs=6)
                            nc.sync.dma_start(qt_sb[:],
                                              qt_d[b][128 * h:128 * (h + 1), q0:q0 + QBLK])
                            at_ps = aps.tile([128, QBLK], F32, name=f"at_{b}_{qb}_{h}",
                                             tag="at", bufs=1)
                            den_ps = aps.tile([128, QBLK], F32, name=f"den_{b}_{qb}_{h}",
                                              tag="den", bufs=1)
                            for kb in range(nkb):
                                i = kb - 4 * qb
                                qoff = 128 * i if i > 0 else 0
                                qn = QBLK - qoff
                                sc_ps = aps.tile([128, QBLK], F32, name=f"sc_{b}_{qb}_{h}_{kb}",
                                                 tag="sc", bufs=4)
                                nc.tensor.matmul(sc_ps[:, :qn],
                                                 ktg[kb // 4][:, 128 * (kb % 4):128 * (kb % 4 + 1)],
                                                 qt_sb[:, qoff:], start=True, stop=True)
                                pt_sb = att.tile([128, QBLK], F32R,
                                                 name=f"pt_{b}_{qb}_{h}_{kb}", tag="pt", bufs=4)
                                nc.scalar.activation(pt_sb[:, qoff:], sc_ps[:, :qn],
                                                     mybir.ActivationFunctionType.Exp)
                                if i >= 0:
                                    nc.vector.tensor_mul(
                                        pt_sb[:, qoff:], pt_sb[:, qoff:],
                                        mask_sb[:, QBLK * i + qoff:QBLK * (i + 1)])
                                nc.tensor.matmul(at_ps[:, qoff:], vg[kb // 4][:, kb % 4, :],
                                                 pt_sb[:, qoff:],
                                                 start=(kb == 0), stop=(kb == nkb - 1))
                                nc.tensor.matmul(den_ps[:, qoff:], ones_sb[:],
                                                 pt_sb[:, qoff:],
                                                 start=(kb == 0), stop=(kb == nkb - 1))
                            recip = att.tile([128, QBLK], F32, name=f"rc_{b}_{qb}_{h}",
                                             tag="rc", bufs=1)
                            nc.vector.reciprocal_approx_fast(recip[:], den_ps[:])
                            an_sb = att.tile([128, QBLK], BF16, name=f"an_{b}_{qb}_{h}",
                                             tag="an", bufs=2)
                            nc.vector.tensor_mul(an_sb[:], at_ps[:], recip[:])
                            nc.gpsimd.dma_start(agin_d[(b, qb)][128 * h:128 * (h + 1), :],
                                                an_sb[:])
                        nc.gpsimd.collective_compute(
                            "AllGather",
                            mybir.AluOpType.bypass,
                            replica_groups=[list(range(N_CORES))],
                            ins=[agin_d[(b, qb)].opt()],
                            outs=[agout_d[(b, qb)].opt()],
                        )
                        if post and qb in post:
                            for fn in post[qb]:
                                fn()

            # pipeline: proj(0) | attn(0) | proj(1) w/ wo(0,*) | attn(1) | wo(1,*)
            with (
                tc.tile_pool(name="att", bufs=1) as att_pool,
                tc.tile_pool(name="aps", bufs=1, space="PSUM") as aps_pool,
            ):
                proj_phase(0, interleave={0: [load_wo]})
                dummy_sb = consts.tile([N_CORES, 64], F32, name="dummy_sb")
                nc.gpsimd.dma_start(dummy_sb[:], dummy_out[:])
                nc.gpsimd.dma_start(dummy_bk[:], dummy_sb[0:1, :])
                att_phase(0)
                proj_phase(1, interleave={
                    0: [lambda: wo_prefetch(0, 0), lambda: wo_prefetch(0, 1),
                        lambda: wo_chunk(0, 0)],
                    2: [lambda: wo_prefetch(0, 2), lambda: wo_chunk(0, 1)],
                    4: [lambda: wo_prefetch(0, 3), lambda: wo_chunk(0, 2)],
                    6: [lambda: wo_chunk(0, 3)],
                })
                att_phase(1, post={
                    1: [lambda: wo_prefetch(1, 0)],
                    2: [lambda: wo_prefetch(1, 1)],
                    3: [lambda: wo_chunk(1, 0), lambda: wo_prefetch(1, 2),
                        lambda: wo_chunk(1, 1), lambda: wo_prefetch(1, 3),
                        lambda: wo_chunk(1, 2), lambda: wo_chunk(1, 3)],
                })
    nc.compile()
    return nc


_NC_CACHE = None
_LAST_IN_MAPS = None


def kernel(x, freqs_cos, freqs_sin, Wq, Wk, Wv, Wo):
    global _NC_CACHE, _LAST_IN_MAPS
    x = np.asarray(x, dtype=np.float32)
    freqs_cos = np.asarray(freqs_cos, dtype=np.float32)
    freqs_sin = np.asarray(freqs_sin, dtype=np.float32)
    Wq = np.asarray(Wq, dtype=np.float32)
    Wk = np.asarray(Wk, dtype=np.float32)
    Wv = np.asarray(Wv, dtype=np.float32)
    Wo = np.asarray(Wo, dtype=np.float32)

    # de-interleave rope pairs within each head: [0,2,...,126, 1,3,...,127]
    perm = np.concatenate([np.arange(0, HD, 2), np.arange(1, HD, 2)])
    scale = 1.0 / math.sqrt(HD)

    xt = x.transpose(0, 2, 1).astype(ml_dtypes.bfloat16)       # [B, D, S]

    cosf = np.empty((128, S), np.float32)
    sinf = np.empty((128, S), np.float32)
    cosf[0:64] = freqs_cos.T
    cosf[64:128] = freqs_cos.T
    sinf[0:64] = -freqs_sin.T
    sinf[64:128] = freqs_sin.T

    mk = np.zeros((4, 128, QBLK), np.float32)
    for i in range(4):
        k_idx = np.arange(128)[:, None]
        j_idx = np.arange(QBLK)[None, :]
        mk[i] = (j_idx >= k_idx + 128 * i).astype(np.float32)

    ident = np.eye(128, dtype=np.float32)
    ones = np.ones((128, 128), np.float32)

    in_maps = []
    for c in range(N_CORES):
        wq_c = Wq[:, 512 * c:512 * (c + 1)].reshape(D, HL, HD)[:, :, perm].reshape(D, HL * HD)
        wk_c = Wk[:, HD * c:HD * (c + 1)][:, perm]
        wv_c = Wv[:, HD * c:HD * (c + 1)]
        wo_c = Wo[:, 512 * c:512 * (c + 1)]
        in_maps.append({
            "xt": xt,
            "wq": (wq_c * scale).astype(ml_dtypes.bfloat16),
            "wk": wk_c.astype(ml_dtypes.bfloat16),
            "wv": wv_c.astype(ml_dtypes.bfloat16),
            "wo": wo_c.astype(ml_dtypes.bfloat16),
            "cosf": cosf.astype(ml_dtypes.bfloat16),
            "sinf": sinf.astype(ml_dtypes.bfloat16),
            "masks": mk.astype(ml_dtypes.bfloat16),
            "ident": ident,
            "ones": ones,
        })

    if _NC_CACHE is None:
        _NC_CACHE = build()
    _LAST_IN_MAPS = in_maps
    res = run_bass_kernel_spmd(_NC_CACHE, in_maps, core_ids=list(range(N_CORES)))

    full = np.empty((B, S, D), np.float32)
    for c in range(N_CORES):
        oc = res.results[c]["out"]                             # [512, B*S]
        full[:, :, 512 * c:512 * (c + 1)] = oc.reshape(512, B, S).transpose(1, 2, 0)
    return full


# revision 31
# speedup vs baseline: 1.0250x; 1.0203x over previous
"""GQA attention (B=2,S=2048,D=4096,H=32,KVH=8,HD=128, RoPE, causal) on 8 TRN2 cores.

Sharding: tensor-parallel over heads. Core c owns q-heads 4c..4c+3 and kv-head c.
Per core: QKV projections (bf16 x / bf16 weights -> f32 psum), RoPE (f32),
transposed-scores causal attention in f32r (scores^T = K^T-tiles x Q^T so the
softmax denominator comes from a ones-matmul and P^T feeds PV directly),
pipelined AllGather of normalized attention output (bf16, one AG per
(batch, 512-query chunk)), and the core's 512-column shard of Wo (bf16)
interleaved with the other batch's projections / later attention chunks.
Host concatenates the 8 column shards.

Self-contained: hardcodes all shapes; no file reads.
"""
import math

import numpy as np
import ml_dtypes

import concourse.mybir as mybir
import concourse.tile as tile
from concourse import bacc
from concourse.bass_utils import run_bass_kernel_spmd

N_CORES = 8
B, S, D = 2, 2048, 4096
H, KVH, HD = 32, 8, 128
HL = H // N_CORES          # 4 local q heads
TOK = B * S

F32 = mybir.dt.float32
F32R = mybir.dt.float32r
BF16 = mybir.dt.bfloat16

TBLK = 256                 # projection token block
QBLK = 512                 # attention query block / AG chunk / Wo token block
KC = D // 128              # 32 contraction chunks
NTB = S // TBLK            # 8 token blocks per batch
NQB = S // QBLK            # 4 query blocks per batch


def tf32_round(x: np.ndarray) -> np.ndarray:
    u = np.ascontiguousarray(x, dtype=np.float32).view(np.uint32)
    r = (u + np.uint32(0x0FFF) + ((u >> np.uint32(13)) & np.uint32(1))) & np.uint32(0xFFFFE000)
    return r.view(np.float32)


def build():
    nc = bacc.Bacc("TRN2", target_bir_lowering=False, debug=False, num_devices=N_CORES)

    xt = nc.declare_dram_parameter("xt", [B, D, S], BF16, isOutput=False)
    wq = nc.declare_dram_parameter("wq", [D, HL * HD], BF16, isOutput=False)
    wk = nc.declare_dram_parameter("wk", [D, HD], BF16, isOutput=False)
    wv = nc.declare_dram_parameter("wv", [D, HD], BF16, isOutput=False)
    wo = nc.declare_dram_parameter("wo", [D, 512], BF16, isOutput=False)
    cosf = nc.declare_dram_parameter("cosf", [128, S], BF16, isOutput=False)
    sinf = nc.declare_dram_parameter("sinf", [128, S], BF16, isOutput=False)
    masks = nc.declare_dram_parameter("masks", [4, 128, QBLK], BF16, isOutput=False)
    ident = nc.declare_dram_parameter("ident", [128, 128], F32R, isOutput=False)
    ones = nc.declare_dram_parameter("ones", [128, 128], F32R, isOutput=False)
    out = nc.declare_dram_parameter("out", [512, TOK], F32, isOutput=True)

    with tile.TileContext(nc) as tc:
        with (
            tc.tile_pool(name="consts", bufs=1) as consts,
            tc.tile_pool(name="wop", bufs=1) as wop,
            tc.tile_pool(name="wps", bufs=1, space="PSUM") as wps,
            tc.tile_pool(name="dram", bufs=1, space="DRAM") as dram,
        ):
            dummy_in = dram.tile([1, 64], F32, name="dummy_in")
            dummy_out = dram.tile([N_CORES, 64], F32, name="dummy_out", addr_space="Shared")
            dummy_bk = dram.tile([1, 64], F32, name="dummy_bk")
            nc.gpsimd.collective_compute(
                "AllGather", mybir.AluOpType.bypass,
                replica_groups=[list(range(N_CORES))],
                ins=[dummy_in.opt()], outs=[dummy_out.opt()])
            cos_sb = consts.tile([128, S], BF16)
            sin_sb = consts.tile([128, S], BF16)
            mask_sb = consts.tile([128, 4 * QBLK], BF16)
            id_sb = consts.tile([128, 128], F32R)
            ones_sb = consts.tile([128, 128], F32R)
            nc.scalar.dma_start(cos_sb[:], cosf[:])
            nc.scalar.dma_start(sin_sb[:], sinf[:])
            nc.scalar.dma_start(mask_sb.rearrange("p (i n) -> p i n", i=4),
                              masks.rearrange("i p n -> p i n"))
            nc.scalar.dma_start(id_sb[:], ident[:])
            nc.scalar.dma_start(ones_sb[:], ones[:])
            wo_sb = wop.tile([128, KC, 512], BF16)
            wq_sb = wop.tile([128, KC, HL * HD], BF16, name="wq_sb")
            wk_sb = wop.tile([128, KC, HD], BF16, name="wk_sb")
            wv_sb = wop.tile([128, KC, HD], BF16, name="wv_sb")
            def load_wq_quarter(q4):
                if q4 == 0:
                    for e in range(4):
                        cs = slice(2 * e, 2 * (e + 1))
                        rs = slice(256 * e, 256 * (e + 1))
                        nc.sync.dma_start(wq_sb[:, cs, :],
                                          wq[rs, :].rearrange("(c p) m -> p c m", p=128))
                else:
                    cs = slice(8 * q4, 8 * (q4 + 1))
                    rs = slice(1024 * q4, 1024 * (q4 + 1))
                    nc.sync.dma_start(wq_sb[:, cs, :],
                                      wq[rs, :].rearrange("(c p) m -> p c m", p=128))
                cs = slice(8 * q4, 8 * (q4 + 1))
                rs = slice(1024 * q4, 1024 * (q4 + 1))
                nc.gpsimd.dma_start(wk_sb[:, cs, :],
                                    wk[rs, :].rearrange("(c p) m -> p c m", p=128))
                nc.gpsimd.dma_start(wv_sb[:, cs, :],
                                    wv[rs, :].rearrange("(c p) m -> p c m", p=128))

            def load_wo():
                for q4 in range(4):
                    cs = slice(8 * q4, 8 * (q4 + 1))
                    rs = slice(1024 * q4, 1024 * (q4 + 1))
                    nc.sync.dma_start(wo_sb[:, cs, :],
                                      wo[rs, :].rearrange("(c p) m -> p c m", p=128))

            qt_d = [dram.tile([HL * 128, S], F32R, name=f"qt_d{b}") for b in range(B)]
            kt_d = [dram.tile([128, S], F32R, name=f"kt_d{b}") for b in range(B)]
            v_d = [dram.tile([S, 128], F32R, name=f"v_d{b}") for b in range(B)]
            CH = [(b, qb) for b in range(B) for qb in range(NQB)]
            agin_d = {c: dram.tile([512, QBLK], BF16, name=f"agin_{c[0]}_{c[1]}") for c in CH}
            agout_d = {c: dram.tile([512 * N_CORES, QBLK], BF16, name=f"agout_{c[0]}_{c[1]}",
                                    addr_space="Shared") for c in CH}

            rj_cache = {}

            def wo_prefetch(b, qb):
                rjs = []
                for jg in range(KC // 4):
                    rj = wop.tile([128, 4, 512], BF16, name=f"rj_{b}_{qb}_{jg}",
                                  tag="rj", bufs=9)
                    nc.sync.dma_start(
                        rj[:],
                        agout_d[(b, qb)][512 * jg:512 * (jg + 1), :]
                        .rearrange("(i p) t -> p i t", p=128))
                    rjs.append(rj)
                rj_cache[(b, qb)] = rjs

            def wo_chunk(b, qb):
                rjs = rj_cache.pop((b, qb))
                t0 = qb * QBLK
                for dc in range(4):
                    wo_ps = wps.tile([128, 512], F32, name=f"wo_{b}_{qb}_{dc}",
                                     tag="wo", bufs=2)
                    for jc in range(KC):
                        nc.tensor.matmul(wo_ps[:], wo_sb[:, jc, 128 * dc:128 * (dc + 1)],
                                         rjs[jc // 4][:, jc % 4, :],
                                         start=(jc == 0), stop=(jc == KC - 1))
                    osb = wop.tile([128, 512], F32, name=f"o_{b}_{qb}_{dc}",
                                   tag="osb", bufs=2)
                    nc.scalar.copy(osb[:], wo_ps[:])
                    nc.scalar.dma_start(out[128 * dc:128 * (dc + 1),
                                            b * S + t0:b * S + t0 + 512], osb[:])

            def proj_phase(b, interleave=None):
                # interleave: dict tb -> list of callables emitted after that tblk
                with (
                    tc.tile_pool(name=f"xtp{b}", bufs=1) as xtp,
                    tc.tile_pool(name=f"pevac{b}", bufs=1) as pevac,
                ):
                    pps = aps_pool
                    for tb in range(NTB):
                        t0 = tb * TBLK
                        xg = []
                        for g in range(KC // 4):
                            if b == 0 and tb == 0 and g % 2 == 0:
                                load_wq_quarter(g // 2)
                            xt_t = xtp.tile([128, 4, TBLK], BF16, name=f"xt_{b}_{tb}_{g}",
                                            tag="xt", bufs=9)
                            eng = nc.sync if tb < 2 else nc.gpsimd
                            if b == 0 and tb == 0 and g == 0:
                                for ii in range(4):
                                    eng.dma_start(
                                        xt_t[:, ii, :],
                                        xt[0, 128 * ii:128 * (ii + 1), t0:t0 + TBLK])
                            else:
                                eng.dma_start(
                                    xt_t[:],
                                    xt[b, 512 * g:512 * (g + 1), t0:t0 + TBLK]
                                    .rearrange("(i p) t -> p i t", p=128))
                            xg.append(xt_t)

                        def xts(c):
                            return xg[c // 4][:, c % 4, :]

                        def proj_rope(w_sb, h, dst, dst_sl):
                            ps_t = pps.tile([128, TBLK], F32, name=f"ps_{b}_{tb}_{h}",
                                            tag="sc", bufs=4)
                            for c in range(KC):
                                nc.tensor.matmul(ps_t[:], w_sb[:, c, 128 * h:128 * (h + 1)],
                                                 xts(c), start=(c == 0), stop=(c == KC - 1))
                            ev = pevac.tile([128, TBLK], F32, name=f"ev_{b}_{tb}_{h}",
                                            tag="ev", bufs=3)
                            nc.scalar.copy(ev[:], ps_t[:])
                            rot = pevac.tile([128, TBLK], F32, name=f"rot_{b}_{tb}_{h}",
                                             tag="rot", bufs=3)
                            nc.scalar.copy(rot[0:64, :], ev[64:128, :])
                            nc.scalar.copy(rot[64:128, :], ev[0:64, :])
                            nc.vector.tensor_mul(ev[:], ev[:], cos_sb[:, t0:t0 + TBLK])
                            nc.vector.tensor_mul(rot[:], rot[:], sin_sb[:, t0:t0 + TBLK])
                            ro = pevac.tile([128, TBLK], F32R, name=f"ro_{b}_{tb}_{h}",
                                            tag="ro", bufs=3)
                            nc.vector.tensor_add(ro[:], ev[:], rot[:])
                            nc.gpsimd.dma_start(dst[dst_sl], ro[:])

                        for h in range(HL):
                            proj_rope(wq_sb, h, qt_d[b],
                                      (slice(128 * h, 128 * (h + 1)), slice(t0, t0 + TBLK)))
                        proj_rope(wk_sb, 0, kt_d[b], (slice(0, 128), slice(t0, t0 + TBLK)))

                        ps_v = pps.tile([128, TBLK], F32, name=f"psv_{b}_{tb}", tag="sc", bufs=4)
                        for c in range(KC):
                            nc.tensor.matmul(ps_v[:], wv_sb[:, c, :], xts(c),
                                             start=(c == 0), stop=(c == KC - 1))
                        vt_sb = pevac.tile([128, TBLK], F32R, name=f"vt_{b}_{tb}",
                                           tag="vt", bufs=3)
                        nc.scalar.copy(vt_sb[:], ps_v[:])
                        for i in range(TBLK // 128):
                            ps_tr = pps.tile([128, 128], F32R, name=f"ptr_{b}_{tb}_{i}",
                                             tag=("at" if i % 2 == 0 else "den"), bufs=1)
                            nc.tensor.transpose(ps_tr[:], vt_sb[:, 128 * i:128 * (i + 1)],
                                                id_sb[:])
                            vn = pevac.tile([128, 128], F32R, name=f"vn_{b}_{tb}_{i}",
                                            tag="vn", bufs=4)
                            nc.vector.tensor_copy(vn[:], ps_tr[:])
                            nc.gpsimd.dma_start(v_d[b][t0 + 128 * i:t0 + 128 * (i + 1), :],
                                                vn[:])

                        if interleave and tb in interleave:
                            for fn in interleave[tb]:
                                fn()

            def att_phase(b, post=None):
                # post: dict qb -> list of callables emitted after that chunk
                if True:
                    att, aps = att_pool, aps_pool
                    ktg = []
                    for q4 in range(4):
                        kt_t = att.tile([128, 512], F32R, name=f"kt_{b}_{q4}",
                                        tag="kt", bufs=4)
                        nc.sync.dma_start(kt_t[:], kt_d[b][:, 512 * q4:512 * (q4 + 1)])
                        ktg.append(kt_t)
                    vg = []
                    for g in range(4):
                        v_sb = att.tile([128, 4, 128], F32R, name=f"v_{b}_{g}", tag="v", bufs=4)
                        nc.sync.dma_start(
                            v_sb[:],
                            v_d[b][512 * g:512 * (g + 1), :].rearrange("(i p) t -> p i t", p=128))
                        vg.append(v_sb)

                    for qb in range(NQB):
                        q0 = qb * QBLK
                        nkb = 4 * qb + 4
                        for h in range(HL):
                            qt_sb = att.tile([128, QBLK], F32R, name=f"qt_{b}_{qb}_{h}",
                                             tag="qt", bufs=6)
                            nc.sync.dma_start(qt_sb[:],
                                              qt_d[b][128 * h:128 * (h + 1), q0:q0 + QBLK])
                            at_ps = aps.tile([128, QBLK], F32, name=f"at_{b}_{qb}_{h}",
                                             tag="at", bufs=1)
                            den_ps = aps.tile([128, QBLK], F32, name=f"den_{b}_{qb}_{h}",
                                              tag="den", bufs=1)
                            for kb in range(nkb):
                                i = kb - 4 * qb
                                qoff = 128 * i if i > 0 else 0
                                qn = QBLK - qoff
                                sc_ps = aps.tile([128, QBLK], F32, name=f"sc_{b}_{qb}_{h}_{kb}",
                                                 tag="sc", bufs=4)
                                nc.tensor.matmul(sc_ps[:, :qn],
                                                 ktg[kb // 4][:, 128 * (kb % 4):128 * (kb % 4 + 1)],
                                                 qt_sb[:, qoff:], start=True, stop=True)
                                pt_sb = att.tile([128, QBLK], F32R,
                                                 name=f"pt_{b}_{qb}_{h}_{kb}", tag="pt", bufs=5)
                                nc.scalar.activation(pt_sb[:, qoff:], sc_ps[:, :qn],
                                                     mybir.ActivationFunctionType.Exp)
                                if i >= 0:
                                    nc.vector.tensor_mul(
                                        pt_sb[:, qoff:], pt_sb[:, qoff:],
                                        mask_sb[:, QBLK * i + qoff:QBLK * (i + 1)])
                                nc.tensor.matmul(at_ps[:, qoff:], vg[kb // 4][:, kb % 4, :],
                                                 pt_sb[:, qoff:],
                                                 start=(kb == 0), stop=(kb == nkb - 1))
                                nc.tensor.matmul(den_ps[:, qoff:], ones_sb[:],
                                                 pt_sb[:, qoff:],
                                                 start=(kb == 0), stop=(kb == nkb - 1))
                            recip = att.tile([128, QBLK], F32, name=f"rc_{b}_{qb}_{h}",
                                             tag="rc", bufs=1)
                            nc.vector.reciprocal_approx_fast(recip[:], den_ps[:])
                            an_sb = att.tile([128, QBLK], BF16, name=f"an_{b}_{qb}_{h}",
                                             tag="an", bufs=2)
                            nc.vector.tensor_mul(an_sb[:], at_ps[:], recip[:])
                            nc.gpsimd.dma_start(agin_d[(b, qb)][128 * h:128 * (h + 1), :],
                                                an_sb[:])
                        nc.gpsimd.collective_compute(
                            "AllGather",
                            mybir.AluOpType.bypass,
                            replica_groups=[list(range(N_CORES))],
                            ins=[agin_d[(b, qb)].opt()],
                            outs=[agout_d[(b, qb)].opt()],
                        )
                        if post and qb in post:
                            for fn in post[qb]:
                                fn()

            # pipeline: proj(0) | attn(0) | proj(1) w/ wo(0,*) | attn(1) | wo(1,*)
            with (
                tc.tile_pool(name="att", bufs=1) as att_pool,
                tc.tile_pool(name="aps", bufs=1, space="PSUM") as aps_pool,
            ):
                proj_phase(0, interleave={0: [load_wo]})
                dummy_sb = consts.tile([N_CORES, 64], F32, name="dummy_sb")
                nc.gpsimd.dma_start(dummy_sb[:], dummy_out[:])
                nc.gpsimd.dma_start(dummy_bk[:], dummy_sb[0:1, :])
                att_phase(0)
                proj_phase(1, interleave={
                    0: [lambda: wo_prefetch(0, 0), lambda: wo_prefetch(0, 1),
                        lambda: wo_chunk(0, 0)],
                    2: [lambda: wo_prefetch(0, 2), lambda: wo_chunk(0, 1)],
                    4: [lambda: wo_prefetch(0, 3), lambda: wo_chunk(0, 2)],
                    6: [lambda: wo_chunk(0, 3)],
                })
                att_phase(1, post={
                    1: [lambda: wo_prefetch(1, 0)],
                    2: [lambda: wo_prefetch(1, 1)],
                    3: [lambda: wo_chunk(1, 0), lambda: wo_prefetch(1, 2),
                        lambda: wo_chunk(1, 1), lambda: wo_prefetch(1, 3),
                        lambda: wo_chunk(1, 2), lambda: wo_chunk(1, 3)],
                })
    nc.compile()
    return nc


_NC_CACHE = None
_LAST_IN_MAPS = None


def kernel(x, freqs_cos, freqs_sin, Wq, Wk, Wv, Wo):
    global _NC_CACHE, _LAST_IN_MAPS
    x = np.asarray(x, dtype=np.float32)
    freqs_cos = np.asarray(freqs_cos, dtype=np.float32)
    freqs_sin = np.asarray(freqs_sin, dtype=np.float32)
    Wq = np.asarray(Wq, dtype=np.float32)
    Wk = np.asarray(Wk, dtype=np.float32)
    Wv = np.asarray(Wv, dtype=np.float32)
    Wo = np.asarray(Wo, dtype=np.float32)

    # de-interleave rope pairs within each head: [0,2,...,126, 1,3,...,127]
    perm = np.concatenate([np.arange(0, HD, 2), np.arange(1, HD, 2)])
    scale = 1.0 / math.sqrt(HD)

    xt = x.transpose(0, 2, 1).astype(ml_dtypes.bfloat16)       # [B, D, S]

    cosf = np.empty((128, S), np.float32)
    sinf = np.empty((128, S), np.float32)
    cosf[0:64] = freqs_cos.T
    cosf[64:128] = freqs_cos.T
    sinf[0:64] = -freqs_sin.T
    sinf[64:128] = freqs_sin.T

    mk = np.zeros((4, 128, QBLK), np.float32)
    for i in range(4):
        k_idx = np.arange(128)[:, None]
        j_idx = np.arange(QBLK)[None, :]
        mk[i] = (j_idx >= k_idx + 128 * i).astype(np.float32)

    ident = np.eye(128, dtype=np.float32)
    ones = np.ones((128, 128), np.float32)

    in_maps = []
    for c in range(N_CORES):
        wq_c = Wq[:, 512 * c:512 * (c + 1)].reshape(D, HL, HD)[:, :, perm].reshape(D, HL * HD)
        wk_c = Wk[:, HD * c:HD * (c + 1)][:, perm]
        wv_c = Wv[:, HD * c:HD * (c + 1)]
        wo_c = Wo[:, 512 * c:512 * (c + 1)]
        in_maps.append({
            "xt": xt,
            "wq": (wq_c * scale).astype(ml_dtypes.bfloat16),
            "wk": wk_c.astype(ml_dtypes.bfloat16),
            "wv": wv_c.astype(ml_dtypes.bfloat16),
            "wo": wo_c.astype(ml_dtypes.bfloat16),
            "cosf": cosf.astype(ml_dtypes.bfloat16),
            "sinf": sinf.astype(ml_dtypes.bfloat16),
            "masks": mk.astype(ml_dtypes.bfloat16),
            "ident": ident,
            "ones": ones,
        })

    if _NC_CACHE is None:
        _NC_CACHE = build()
    _LAST_IN_MAPS = in_maps
    res = run_bass_kernel_spmd(_NC_CACHE, in_maps, core_ids=list(range(N_CORES)))

    full = np.empty((B, S, D), np.float32)
    for c in range(N_CORES):
        oc = res.results[c]["out"]                             # [512, B*S]
        full[:, :, 512 * c:512 * (c + 1)] = oc.reshape(512, B, S).transpose(1, 2, 0)
    return full
